# revision 1
# baseline (speedup 1.0000x reference)
"""PriorLSTM Trainium2 kernel (8 NeuronCores, SPMD).

Model: BatchNorm1d(IN) -> 16-layer LSTM(H=128) -> Linear(H->OUT) -> max over T
       -> + prior logits.   B=16, T=32, IN=52686, OUT=2976.

Strategy:
  Phase A (tensor-parallel on IN): each core owns 6656 channels (padded).
    BN folded to per-channel scale/shift; big GEMM gx0.T[g,tok] accumulated
    over 52 K-chunks of 128 channels; AllReduce of the [512,512] partial.
  Phase B (layer pipeline): stage c = layers (2c, 2c+1). States transposed:
    h.T/c.T = [128 h-dim, 16 batch]; weights (pre-transposed on host) are
    the matmul stationary so gates come out as gates.T [128g-block, 16].
    tanh via sigmoid identity (host pre-scales g-gate rows by 2).
    Boundary h crosses cores via one small AllGather per timestep; each
    core selects its predecessor's slice with a 0/1 mask (SPMD-uniform).
    Per-core validity handled by per-tick reset masks from input data.
  Phase C: output projection + temporal max-pool; b_out and prior logits
    folded host-side into one additive constant.

Tokens are time-major: tok = t*16 + b.
"""

import numpy as np

B, T, IN, H, L, OUT = 16, 32, 52686, 128, 16, 2976
EPS = 1e-5
NC = 8
INL = 6656          # channels per core (padded)
NCH = INL // 128    # 52 K-chunks per core
INP = INL * NC      # 53248
TOK = B * T         # 512
OUTP = 3072         # padded OUT
NT = T + NC         # pipeline ticks = 40
NSL = NT + 1        # hist slots


def build_kernel():
    import concourse.bass as bass
    import concourse.bacc as bacc
    import concourse.mybir as mybir
    import concourse.tile as tile

    f32 = mybir.dt.float32
    Alu = mybir.AluOpType
    Act = mybir.ActivationFunctionType

    nc = bacc.Bacc(None, num_devices=NC)

    # ---------------- inputs ------------------------------------------------
    xT = nc.dram_tensor("xT", [INL, TOK], f32, kind="ExternalInput")
    w0T = nc.dram_tensor("w0T", [INL, 512], f32, kind="ExternalInput")
    gam = nc.dram_tensor("gam", [128, NCH], f32, kind="ExternalInput")
    bet = nc.dram_tensor("bet", [128, NCH], f32, kind="ExternalInput")
    wihT = nc.dram_tensor("wihT", [128, 2 * 512], f32, kind="ExternalInput")
    whhT = nc.dram_tensor("whhT", [128, 2 * 512], f32, kind="ExternalInput")
    # btile1[p, gb*16+b] = bias_{layer 2c+1}[gb*128+p]  (broadcast over b)
    btile1 = nc.dram_tensor("btile1", [128, 64], f32, kind="ExternalInput")
    # bcol[p, gb] = bias of the stage's bottom layer (goes into inj)
    bcol = nc.dram_tensor("bcol", [128, 4], f32, kind="ExternalInput")
    m_inj = nc.dram_tensor("m_inj", [128, 1], f32, kind="ExternalInput")
    msel = nc.dram_tensor("msel", [128, NC], f32, kind="ExternalInput")
    mrt = nc.dram_tensor("mrt", [128, NT * 32], f32, kind="ExternalInput")
    woutT = nc.dram_tensor("woutT", [128, OUTP], f32, kind="ExternalInput")
    padd = nc.dram_tensor("padd", [16, OUTP], f32, kind="ExternalInput")

    outp = nc.dram_tensor("outp", [16, OUTP], f32, kind="ExternalOutput")

    with tile.TileContext(nc) as tc:
        with (
            tc.tile_pool(name="big", bufs=1) as big,
            tc.tile_pool(name="wstream", bufs=3) as wst,
            tc.tile_pool(name="small", bufs=2) as small,
            tc.tile_pool(name="ew", bufs=3) as ew,
            tc.tile_pool(name="dram", bufs=4, space="DRAM") as dpool,
        ):
            # ---------------- phase A --------------------------------------
            xtsp_cm = tc.tile_pool(name="xtsp", bufs=1)
            xtsp = xtsp_cm.__enter__()
            SLAB = 13
            xts_l = [xtsp.tile([128, SLAB * TOK], f32, tag=f"xts{s}",
                               name=f"xts{s}") for s in range(4)]

            def xchunk(ch):
                return xts_l[ch // SLAB][
                    :, (ch % SLAB) * TOK:(ch % SLAB + 1) * TOK]
            sums = big.tile([128, NCH], f32, tag="sums")
            sumsq = big.tile([128, NCH], f32, tag="sumsq")
            gams = big.tile([128, NCH], f32, tag="gams")
            bets = big.tile([128, NCH], f32, tag="bets")

            nc.sync.dma_start(out=gams[:], in_=gam[:])
            nc.sync.dma_start(out=bets[:], in_=bet[:])

            xview = xT.rearrange("(c p) t -> p c t", p=128)
            for s in range(4):
                nc.sync.dma_start(
                    out=xts_l[s][:].rearrange("p (c t) -> p c t", t=TOK),
                    in_=xview[:, s * SLAB:(s + 1) * SLAB, :],
                )
            for ch in range(NCH):
                xc = xchunk(ch)
                scr = small.tile([128, TOK], f32, tag="scr")
                nc.vector.tensor_reduce(
                    sums[:, ch:ch + 1], xc, mybir.AxisListType.X, Alu.add,
                )
                nc.vector.scalar_tensor_tensor(
                    out=scr[:], in0=xc, scalar=1.0, in1=xc,
                    op0=Alu.mult, op1=Alu.mult,
                    accum_out=sumsq[:, ch:ch + 1],
                )
            mean = big.tile([128, NCH], f32, tag="mean")
            var = big.tile([128, NCH], f32, tag="var")
            sd = big.tile([128, NCH], f32, tag="sd")
            r0 = big.tile([128, NCH], f32, tag="r0")
            t1 = big.tile([128, NCH], f32, tag="t1")
            scl = big.tile([128, NCH], f32, tag="scl")
            b2 = big.tile([128, NCH], f32, tag="b2")
            nc.vector.tensor_scalar_mul(mean[:], sums[:], 1.0 / TOK)
            nc.vector.tensor_scalar_mul(var[:], sumsq[:], 1.0 / TOK)
            nc.vector.scalar_tensor_tensor(
                out=t1[:], in0=mean[:], scalar=-1.0, in1=mean[:],
                op0=Alu.mult, op1=Alu.mult)
            nc.vector.tensor_tensor(var[:], var[:], t1[:], Alu.add)
            nc.vector.tensor_scalar_add(var[:], var[:], EPS)
            nc.scalar.activation(sd[:], var[:], Act.Sqrt)
            nc.vector.reciprocal(r0[:], sd[:])
            nc.vector.tensor_tensor(t1[:], r0[:], r0[:], Alu.mult)
            nc.vector.tensor_tensor(t1[:], t1[:], var[:], Alu.mult)
            nc.vector.tensor_scalar(
                out=t1[:], in0=t1[:], scalar1=-0.5, scalar2=1.5,
                op0=Alu.mult, op1=Alu.add)
            nc.vector.tensor_tensor(r0[:], r0[:], t1[:], Alu.mult)
            nc.vector.tensor_tensor(scl[:], r0[:], gams[:], Alu.mult)
            nc.vector.scalar_tensor_tensor(
                out=b2[:], in0=mean[:], scalar=-1.0, in1=scl[:],
                op0=Alu.mult, op1=Alu.mult)
            nc.vector.tensor_tensor(b2[:], b2[:], bets[:], Alu.add)

            with tc.tile_pool(name="psgx", bufs=1, space="PSUM") as psgx:
                gx = [psgx.tile([128, TOK], f32, name=f"gx{gb}", tag=f"gx{gb}")
                      for gb in range(4)]
                for ch in range(NCH):
                    xc = xchunk(ch)
                    nc.vector.tensor_scalar(
                        out=xc, in0=xc,
                        scalar1=scl[:, ch:ch + 1], scalar2=b2[:, ch:ch + 1],
                        op0=Alu.mult, op1=Alu.add)
                    wc = wst.tile([128, 512], f32, tag="wc")
                    nc.sync.dma_start(
                        out=wc[:], in_=w0T[ch * 128:(ch + 1) * 128, :])
                    for gb in range(4):
                        nc.tensor.matmul(
                            gx[gb][:], wc[:, gb * 128:(gb + 1) * 128], xc,
                            start=(ch == 0), stop=(ch == NCH - 1),
                            skip_group_check=True)

                gx0 = big.tile([128, 4 * TOK], f32, tag="gx0")
                for gb in range(4):
                    nc.vector.tensor_copy(
                        gx0[:, gb * TOK:(gb + 1) * TOK], gx[gb][:])

            xtsp_cm.__exit__(None, None, None)
            arin = dpool.tile([128, 4 * TOK], f32, tag="arin")
            arout = dpool.tile([128, 4 * TOK], f32, tag="arout")
            nc.gpsimd.dma_start(out=arin[:], in_=gx0[:])
            nc.gpsimd.collective_compute(
                "AllReduce", Alu.add,
                ins=[arin[:].opt()], outs=[arout[:].opt()],
                replica_groups=[list(range(NC))],
            )
            nc.sync.dma_start(out=gx0[:], in_=arout[:])

            # ---------------- phase B --------------------------------------
            wih_s = big.tile([128, 2 * 512], f32, tag="wih")
            whh_s = big.tile([128, 2 * 512], f32, tag="whh")
            bt1_s = big.tile([128, 64], f32, tag="bt1")
            mi_s = small.tile([128, 1], f32, tag="mi")
            bc_s = small.tile([128, 4], f32, tag="bc")
            msel_s = small.tile([128, NC], f32, tag="msel")
            mrt_s = big.tile([128, NT * 32], f32, tag="mrt")
            nc.sync.dma_start(out=wih_s[:], in_=wihT[:])
            nc.sync.dma_start(out=whh_s[:], in_=whhT[:])
            nc.sync.dma_start(out=bt1_s[:], in_=btile1[:])
            nc.sync.dma_start(out=mi_s[:], in_=m_inj[:])
            nc.sync.dma_start(out=bc_s[:], in_=bcol[:])
            nc.sync.dma_start(out=msel_s[:], in_=msel[:])
            nc.sync.dma_start(out=mrt_s[:], in_=mrt[:])
            fence = small.tile([128, 8], f32, tag="fence")
            nc.vector.tensor_copy(fence[:, 0:1], mi_s[:, 0:1])
            nc.vector.tensor_copy(fence[:, 1:2], bc_s[:, 0:1])
            nc.vector.tensor_copy(fence[:, 2:3], msel_s[:, 0:1])
            nc.vector.tensor_copy(fence[:, 3:4], bt1_s[:, 0:1])
            nc.vector.tensor_copy(fence[:, 4:5], mrt_s[:, 0:1])

            # inj[gb, t, b] = m_inj * gx0 + bias_bottom   (padded to NT steps)
            nc.vector.tensor_copy(fence[:, 5:6], gx0[:, 0:1])
            inj = big.tile([128, 4 * NT * 16], f32, tag="inj")
            nc.vector.memset(inj[:], 0.0)
            injv = inj[:, :].rearrange("p (g t b) -> p g t b", g=4, b=16)
            gx0v = gx0[:, :].rearrange("p (g t b) -> p g t b", g=4, b=16)
            for gb in range(4):
                nc.vector.tensor_scalar(
                    out=injv[:, gb:gb + 1, 0:T, :],
                    in0=gx0v[:, gb:gb + 1, :, :],
                    scalar1=mi_s[:, 0:1], scalar2=bc_s[:, gb:gb + 1],
                    op0=Alu.mult, op1=Alu.add)
            for gb in range(4):  # bias-only for the padded tail ticks
                nc.vector.tensor_scalar(
                    out=injv[:, gb:gb + 1, T:NT, :],
                    in0=injv[:, gb:gb + 1, T:NT, :],
                    scalar1=0.0, scalar2=bc_s[:, gb:gb + 1],
                    op0=Alu.mult, op1=Alu.add)

            hist = big.tile([128, NSL * 32], f32, tag="hist")
            cst = big.tile([128, 32], f32, tag="cst")
            nc.vector.memset(hist[:, 0:64], 0.0)
            nc.vector.memset(cst[:], 0.0)

            with tc.tile_pool(name="psb", bufs=3, space="PSUM") as psb:
                for m in range(NT):
                    # --- receive/select predecessor boundary h ---
                    hinc = ew.tile([128, 16], f32, tag="hinc")
                    if m == 0:
                        nc.vector.memset(hinc[:], 0.0)
                    else:
                        agsb = ew.tile([128, NC * 16], f32, tag="agsb")
                        nc.sync.dma_start(
                            out=agsb[:].rearrange(
                                "p (r b) -> p r b", r=NC),
                            in_=ag_out.rearrange("(r p) b -> p r b", p=128))
                        agv = agsb[:, :].rearrange("p (r b) -> p r b", r=NC)
                        nc.vector.tensor_scalar_mul(
                            hinc[:], agv[:, 0:1, :], msel_s[:, 0:1])
                        for r in range(1, NC):
                            nc.vector.scalar_tensor_tensor(
                                out=hinc[:], in0=agv[:, r:r + 1, :],
                                scalar=msel_s[:, r:r + 1], in1=hinc[:],
                                op0=Alu.mult, op1=Alu.add)

                    # --- reset masks (zero garbage state before first use) ---
                    sl = hist[:, m * 32:(m + 1) * 32]
                    mm = mrt_s[:, m * 32:(m + 1) * 32]
                    nc.vector.tensor_tensor(sl, sl, mm, Alu.mult)
                    nc.vector.tensor_tensor(cst[:], cst[:], mm, Alu.mult)

                    # --- gate matmuls (k=0: layer 2c, k=1: layer 2c+1) ---
                    zp = []
                    for k in range(2):
                        p = psb.tile([128, 64], f32, tag=f"gps{k}")
                        zp.append(p)
                        xin = hinc[:] if k == 0 else sl[:, 0:16]
                        hprev = sl[:, k * 16:(k + 1) * 16]
                        for gb in range(4):
                            nc.tensor.matmul(
                                p[:, gb * 16:(gb + 1) * 16],
                                wih_s[:, k * 512 + gb * 128:
                                      k * 512 + (gb + 1) * 128], xin,
                                start=True, stop=False,
                                skip_group_check=True)
                        for gb in range(4):
                            nc.tensor.matmul(
                                p[:, gb * 16:(gb + 1) * 16],
                                whh_s[:, k * 512 + gb * 128:
                                      k * 512 + (gb + 1) * 128], hprev,
                                start=False, stop=True,
                                skip_group_check=True)

                    # --- z = gates + bias/injection, stacked [128,128] ---
                    zs = ew.tile([128, 128], f32, tag="zs")
                    nc.vector.tensor_tensor(
                        zs[:, 0:64], zp[0][:],
                        injv[:, :, m:m + 1, :], Alu.add)
                    nc.vector.tensor_tensor(
                        zs[:, 64:128], zp[1][:], bt1_s[:], Alu.add)

                    sg = ew.tile([128, 128], f32, tag="sg")
                    nc.scalar.activation(sg[:], zs[:], Act.Sigmoid)

                    sgv = sg[:, :].rearrange("p (k g b) -> p k g b", k=2, b=16)
                    i_sl = sgv[:, :, 0:1, :]
                    f_sl = sgv[:, :, 1:2, :]
                    g_sl = sgv[:, :, 2:3, :]
                    o_sl = sgv[:, :, 3:4, :]

                    v = ew.tile([128, 32], f32, tag="v")
                    mt = ew.tile([128, 32], f32, tag="mt")
                    nc.vector.tensor_tensor(cst[:], cst[:], f_sl, Alu.mult)
                    nc.vector.tensor_scalar(
                        out=v[:], in0=g_sl, scalar1=2.0, scalar2=-1.0,
                        op0=Alu.mult, op1=Alu.add)
                    nc.vector.tensor_tensor(mt[:], v[:], i_sl, Alu.mult)
                    nc.vector.tensor_tensor(cst[:], cst[:], mt[:], Alu.add)
                    w2 = ew.tile([128, 32], f32, tag="w2")
                    nc.scalar.activation(w2[:], cst[:], Act.Sigmoid, scale=2.0)
                    # h = sigma(o)*(2*sig(2c)-1) = 2*(sig2c*o) - o
                    h1 = ew.tile([128, 32], f32, tag="h1")
                    nc.vector.tensor_tensor(h1[:], w2[:], o_sl, Alu.mult)
                    nc.vector.scalar_tensor_tensor(
                        out=hist[:, (m + 1) * 32:(m + 2) * 32],
                        in0=h1[:], scalar=2.0, in1=o_sl,
                        op0=Alu.mult, op1=Alu.subtract)

                    # --- ship boundary h (layer 2c+1) via AllGather ---
                    if m < NT - 1:
                        ag_in = dpool.tile([128, 16], f32, tag="agi")
                        ag_out = dpool.tile([128 * NC, 16], f32, tag="ago")
                        nc.sync.dma_start(
                            out=ag_in[:],
                            in_=hist[:, (m + 1) * 32 + 16:(m + 2) * 32])
                        nc.gpsimd.collective_compute(
                            "AllGather", Alu.bypass,
                            ins=[ag_in[:].opt()], outs=[ag_out[:].opt()],
                            replica_groups=[list(range(NC))],
                        )

            # ---------------- phase C --------------------------------------
            pcp_cm = tc.tile_pool(name="pcp", bufs=1)
            pcp = pcp_cm.__enter__()
            wout_s = pcp.tile([128, OUTP], f32, tag="wout", name="wout_s")
            padd_s = pcp.tile([16, OUTP], f32, tag="padd", name="padd_s")
            nc.sync.dma_start(out=wout_s[:], in_=woutT[:])
            nc.sync.dma_start(out=padd_s[:], in_=padd[:])
            msb = pcp.tile([128, OUTP], f32, tag="msb", name="msb")
            histv = hist[:, :].rearrange("p (s x) -> p s x", x=32)
            base = NC + 1  # first slot holding a real top-layer h on stage 7
            with tc.tile_pool(name="psc", bufs=2, space="PSUM") as psc:
                for j in range(4):
                    hstg = pcp.tile([128, 128], f32, tag="hstg",
                                    name=f"hstg{j}", bufs=2)
                    nc.vector.tensor_copy(
                        hstg[:],
                        histv[:, base + 8 * j:base + 8 * j + 8, 16:32])
                    lhs = hstg[:]
                    for ob in range(OUTP // 512):
                        pc = psc.tile([128, 512], f32, tag="pc")
                        nc.tensor.matmul(
                            pc[:], lhs, wout_s[:, ob * 512:(ob + 1) * 512],
                            start=True, stop=True, skip_group_check=True)
                        if j == 0:
                            nc.vector.tensor_copy(
                                msb[:, ob * 512:(ob + 1) * 512], pc[:])
                        else:
                            nc.vector.tensor_tensor(
                                msb[:, ob * 512:(ob + 1) * 512], pc[:],
                                msb[:, ob * 512:(ob + 1) * 512], Alu.max)
            f1 = pcp.tile([64, OUTP], f32, tag="f1", name="f1")
            fs = pcp.tile([64, OUTP], f32, tag="fs", name="fs")
            nc.sync.dma_start(out=fs[:], in_=msb[64:128, :])
            nc.vector.tensor_tensor(f1[:], msb[0:64, :], fs[:], Alu.max)
            nc.sync.dma_start(out=fs[0:32, :], in_=f1[32:64, :])
            nc.vector.tensor_tensor(
                f1[0:32, :], f1[0:32, :], fs[0:32, :], Alu.max)
            nc.sync.dma_start(out=fs[0:16, :], in_=f1[16:32, :])
            nc.vector.tensor_tensor(
                f1[0:16, :], f1[0:16, :], fs[0:16, :], Alu.max)
            nc.vector.tensor_tensor(
                f1[0:16, :], f1[0:16, :], padd_s[:], Alu.add)
            nc.sync.dma_start(out=outp[:], in_=f1[0:16, :])
            pcp_cm.__exit__(None, None, None)

    nc.compile()
    return nc


def prep_inputs(x, bn_gamma, bn_beta, W_ih0, W_ih, W_hh, b_ih, b_hh,
                W_out, b_out, prior):
    """Host-side sharding / layout prep. Returns in_maps list."""
    x = np.asarray(x, np.float32)
    pad = INP - IN
    # time-major tokens: [B,T,IN] -> [T,B,IN] -> [TOK, INP] -> transpose
    xtb = np.ascontiguousarray(x.transpose(1, 0, 2).reshape(TOK, IN))
    xtb = np.pad(xtb, ((0, 0), (0, pad)))
    xT_full = np.ascontiguousarray(xtb.T)             # [INP, TOK]

    W0 = np.array(W_ih0, np.float32)
    W0[2 * H:3 * H, :] *= 2.0
    w0T_full = np.ascontiguousarray(np.pad(W0, ((0, 0), (0, pad))).T)

    gp = np.pad(np.asarray(bn_gamma, np.float32), (0, pad))
    bp = np.pad(np.asarray(bn_beta, np.float32), (0, pad))

    bias = (np.asarray(b_ih, np.float32)
            + np.asarray(b_hh, np.float32)).copy()    # [L, 512]
    bias[:, 2 * H:3 * H] *= 2.0
    Wih = np.array(W_ih, np.float32)                  # [L-1, 512, 128]
    Wih[:, 2 * H:3 * H, :] *= 2.0
    Whh = np.array(W_hh, np.float32)                  # [L, 512, 128]
    Whh[:, 2 * H:3 * H, :] *= 2.0

    WoT = np.zeros((128, OUTP), np.float32)
    WoT[:, :OUT] = np.asarray(W_out, np.float32).T

    p = np.clip(np.asarray(prior, np.float64), 1e-8, 1 - 1e-8)
    logit = (np.log(p) - np.log1p(-p)).astype(np.float32)
    paddv = np.zeros((16, OUTP), np.float32)
    paddv[:, :OUT] = np.asarray(b_out, np.float32)[None, :]
    paddv[:, 1:OUT] += logit[None, :]

    in_maps = []
    for c in range(NC):
        sl = slice(c * INL, (c + 1) * INL)
        l0, l1 = 2 * c, 2 * c + 1
        wihT_c = np.zeros((128, 2 * 512), np.float32)
        if l0 >= 1:
            wihT_c[:, 0:512] = Wih[l0 - 1].T
        wihT_c[:, 512:1024] = Wih[l1 - 1].T
        whhT_c = np.concatenate([Whh[l0].T, Whh[l1].T], axis=1)
        bt1 = np.repeat(bias[l1].reshape(4, 128).T[:, :, None], 16,
                        axis=2).reshape(128, 64)
        bc = np.ascontiguousarray(bias[l0].reshape(4, 128).T)
        mselv = np.zeros((128, NC), np.float32)
        if c > 0:
            mselv[:, c - 1] = 1.0
        mrtv = np.zeros((128, NT, 2, 16), np.float32)
        for m in range(NT):
            mrtv[:, m, 0, :] = 1.0 if m > c else 0.0
            mrtv[:, m, 1, :] = 1.0 if m > c + 1 else 0.0
        in_maps.append({
            "xT": np.ascontiguousarray(xT_full[sl]),
            "w0T": np.ascontiguousarray(w0T_full[sl]),
            "gam": np.ascontiguousarray(gp[sl].reshape(NCH, 128).T),
            "bet": np.ascontiguousarray(bp[sl].reshape(NCH, 128).T),
            "wihT": wihT_c,
            "whhT": np.ascontiguousarray(whhT_c),
            "btile1": bt1,
            "bcol": bc,
            "m_inj": np.full((128, 1), 1.0 if c == 0 else 0.0, np.float32),
            "msel": mselv,
            "mrt": np.ascontiguousarray(mrtv.reshape(128, NT * 32)),
            "woutT": WoT,
            "padd": paddv,
        })
    return in_maps


_CACHED = {}


def kernel(**inputs):
    from concourse.bass_utils import run_bass_kernel_spmd

    if "nc" not in _CACHED:
        _CACHED["nc"] = build_kernel()
    nc = _CACHED["nc"]
    in_maps = prep_inputs(**inputs)
    res = run_bass_kernel_spmd(nc, in_maps, core_ids=list(range(NC)))
    _CACHED["res"] = res
    out = res.results[NC - 1]["outp"][:, :OUT]
    return np.ascontiguousarray(out)


if __name__ == "__main__":
    import reference
    inputs = {k: np.asarray(v) for k, v in reference.setup_inputs().items()}
    got = kernel(**inputs)
    exp = np.asarray(reference.reference(**inputs))
    denom = np.abs(exp).max() + 1e-9
    print("Relative error:", np.abs(got - exp).max() / denom)

